# revision 6
# baseline (speedup 1.0000x reference)
"""Trainium2 Bass kernel for nn_AttentionLiereRotator.

Math: skew_params [4, 496, 1024, 2] -> c = einsum('kmad,a->km') -> 4 skew-symmetric
32x32 blocks S -> R = expm(S) (per block) -> out = blockdiag(R) applied along
head_dim of x [4, 2048, 16, 128].

Split of work:
  - The tiny per-block rotation matrices R (O(head_dim^2) data) are computed on
    host, mirroring the reference ops exactly (jax fp32 on CPU) so that the
    result -- including the exact fp32 expm behaviour on these huge-norm skew
    matrices (which NaNs out for randn-scale inputs) -- matches the reference
    bit-for-bit.  They are replicated to all 8 cores (per the sharding hint).
  - The heavy part (the [131072, 128] x [128, 128] block-diagonal rotation,
    ~1 GFLOP / 128 MB of traffic) runs on the 8 NeuronCores, data-parallel
    over the flattened batch*seq*heads token axis.

Per-core device pipeline (16384 tokens = 128 tiles of 128 tokens, grouped into
ramped DMA batches of 4/8 tiles):
  DMA in [128 tok, S, 128 dim] -> PE transpose per 128x128 tile -> PSUM ->
  DVE copy to SBUF -> PE matmul (lhsT = x^T tile, rhs = blockdiag(R)^T, fp32,
  N=128) -> PSUM [tok, dim] -> ACT copy to SBUF -> DMA out.
Input loads go out on the SP HWDGE queue, stores on the GpSimd SWDGE queue so
loads are never stuck behind stores; 16 junk PE transposes at the start warm
the HAM clock while the first load is in flight.

Rows of R that are non-finite (the fp32 reference expm NaNs out on large-norm
inputs) are reproduced exactly on the host afterwards.

Measured (8 cores, fp32): steady-state ~44-48 us per pass on HW (loop
differencing), cost-model single pass ~52 us; DMA-bound (16.5 MB/core at
~358 GB/s); device-path rel err vs fp64 reference ~3.3e-7.
"""

import numpy as np

B, SEQ, HEADS, HEAD_DIM = 4, 2048, 16, 128
BLOCK = 32
NBLK = HEAD_DIM // BLOCK
AXES, SPACIAL = 1024, 2
N_CORES = 8
T = B * SEQ * HEADS            # 131072 tokens
T_CORE = T // N_CORES          # 16384 tokens per core
N_TILES = T_CORE // 128        # 128 tiles of 128 tokens per core
# DMA group schedule (in tiles): small first/last groups for fast pipeline
# fill/drain, 8-tile (512 KB) groups in the middle.
SCHEDULE = [4, 4] + [8] * 14 + [4, 4]
assert sum(SCHEDULE) == N_TILES

_NC_CACHE = {}


def _build_nc(loops=None):
    """Build (and cache) the Bass module for the per-core rotation kernel.

    loops: when given, wraps the whole pipeline in a device-side For_i that
    re-runs it `loops` times (used only for steady-state wall-clock timing;
    kernel() always uses the single-pass module)."""
    if loops is None and "nc" in _NC_CACHE:
        return _NC_CACHE["nc"]

    import contextlib
    import concourse.bass as bass  # noqa: F401  (registers engine namespaces)
    from concourse import bacc, tile, mybir
    from concourse.masks import make_identity

    f32 = mybir.dt.float32
    nc = bacc.Bacc("TRN2", target_bir_lowering=False, debug=False,
                   num_devices=N_CORES)
    x_d = nc.dram_tensor("xs", [N_TILES, 128, 128], f32, kind="ExternalInput")
    r_d = nc.dram_tensor("rbdT", [128, 128], f32, kind="ExternalInput")
    o_d = nc.dram_tensor("out", [N_TILES, 128, 128], f32, kind="ExternalOutput")

    WARMUP = 16  # junk PE transposes to warm the HAM clock during startup DMA

    with tile.TileContext(nc) as tc:
        with tc.tile_pool(name="consts", bufs=1) as consts, \
             tc.tile_pool(name="xin", bufs=5) as xin_pool, \
             tc.tile_pool(name="xt", bufs=4) as xt_pool, \
             tc.tile_pool(name="osb", bufs=4) as out_pool, \
             tc.tile_pool(name="pst", bufs=4, space="PSUM") as psum_t, \
             tc.tile_pool(name="psm", bufs=4, space="PSUM") as psum_m:
            ident = consts.tile([128, 128], f32)
            make_identity(nc, ident[:])
            rbdT = consts.tile([128, 128], f32)
            # rbdT on the ACT HWDGE queue so it doesn't serialize with the
            # first x loads on SP's queue.
            nc.scalar.dma_start(rbdT[:], r_d.ap())
            wp = psum_t.tile([128, 4, 128], f32, tag="xt_ps")
            for w in range(WARMUP):
                nc.tensor.transpose(wp[:, w % 4, :], ident[:], ident[:])
            loop_ctx = (tc.For_i(0, loops, 1) if loops is not None
                        else contextlib.nullcontext())
            with loop_ctx:
                _emit_groups(nc, tc, mybir, x_d, o_d, ident, rbdT,
                             xin_pool, xt_pool, out_pool, psum_t, psum_m)
    nc.finalize()
    if loops is None:
        _NC_CACHE["nc"] = nc
    return nc


def _emit_groups(nc, tc, mybir, x_d, o_d, ident, rbdT,
                 xin_pool, xt_pool, out_pool, psum_t, psum_m):
    f32 = mybir.dt.float32
    t0 = 0
    n_groups = len(SCHEDULE)
    for gi, S in enumerate(SCHEDULE):
        x_sb = xin_pool.tile([128, 8, 128], f32, tag="x_sb")
        src = x_d.ap()[t0:t0 + S].rearrange("s p j -> p s j")
        nc.sync.dma_start(x_sb[:, :S, :], src)
        out_sb = out_pool.tile([128, 8, 128], f32, tag="out_sb")
        for q in range(S // 4):
            xt_ps = psum_t.tile([128, 4, 128], f32, tag="xt_ps")
            for s4 in range(4):
                nc.tensor.transpose(xt_ps[:, s4, :], x_sb[:, q * 4 + s4, :],
                                    ident[:])
            xt_sb = xt_pool.tile([128, 4, 128], f32)
            nc.vector.tensor_copy(xt_sb[:], xt_ps[:])
            o_ps = psum_m.tile([128, 4, 128], f32)
            for s4 in range(4):
                nc.tensor.matmul(o_ps[:, s4, :], xt_sb[:, s4, :], rbdT[:],
                                 start=True, stop=True)
            nc.scalar.copy(out_sb[:, q * 4:(q + 1) * 4, :], o_ps[:])
        # stores on the SWDGE (gpsimd) queue so the next group's load on SP's
        # HWDGE queue is never stuck behind them; the last two (small) groups
        # store via the now-idle SP queue, whose HWDGE first-byte latency is
        # lower, to drain the tail faster.
        dsto = o_d.ap()[t0:t0 + S].rearrange("s p j -> p s j")
        eng = nc.sync if gi >= n_groups - 2 else nc.gpsimd
        eng.dma_start(dsto, out_sb[:, :S, :])
        t0 += S


def _host_rotations(skew_params):
    """Per-block rotation matrices, mirroring the reference computation exactly
    (fp32 jax on CPU): c-contraction, skew-symmetric fill, fp32 expm."""
    import jax
    import jax.numpy as jnp
    from jax.scipy.linalg import expm

    try:
        import contextlib
        cpu = jax.local_devices(backend="cpu")
        ctx = jax.default_device(cpu[0]) if cpu else contextlib.nullcontext()
    except Exception:
        import contextlib
        ctx = contextlib.nullcontext()
    with ctx:
        pos = jnp.arange(AXES, dtype=jnp.float32)
        c = jnp.einsum("kmad,a->km", jnp.asarray(np.asarray(skew_params)), pos)
        i, j = jnp.tril_indices(BLOCK, -1)
        Sm = jnp.zeros((NBLK, BLOCK, BLOCK), dtype=jnp.float32)
        Sm = Sm.at[:, i, j].set(c).at[:, j, i].set(-c)
        R = jax.vmap(expm)(Sm)
        return np.asarray(jax.device_get(R))


def _run_device(x_flat, Rbd):
    """Run the block-diagonal rotation on the 8 NeuronCores.
    x_flat: [T, 128] fp32 contiguous; Rbd: [128, 128] fp32 (finite).
    Returns [T, 128] fp32."""
    from concourse.bass_utils import run_bass_kernel_spmd

    nc = _build_nc()
    shards = x_flat.reshape(N_CORES, N_TILES, 128, 128)
    rbdT = np.ascontiguousarray(Rbd.T)
    in_maps = [{"xs": shards[c], "rbdT": rbdT} for c in range(N_CORES)]
    res = run_bass_kernel_spmd(nc, in_maps, list(range(N_CORES)))
    out = np.empty((N_CORES, T_CORE, HEAD_DIM), np.float32)
    for c in range(N_CORES):
        out[c] = res.results[c]["out"].reshape(T_CORE, HEAD_DIM)
    return out.reshape(T, HEAD_DIM)


def kernel(x, skew_params):
    x = np.asarray(x, dtype=np.float32)
    skew_params = np.asarray(skew_params, dtype=np.float32)

    R = _host_rotations(skew_params)                       # [NBLK, 32, 32] fp32

    # Block-diagonal rotation matrix for the device; zero out non-finite
    # entries (their output rows are reproduced on the host below).
    Rbd = np.zeros((HEAD_DIM, HEAD_DIM), np.float32)
    for k in range(NBLK):
        Rbd[k * BLOCK:(k + 1) * BLOCK, k * BLOCK:(k + 1) * BLOCK] = R[k]
    finite_mask = np.isfinite(Rbd)
    Rbd_dev = np.where(finite_mask, Rbd, np.float32(0.0))

    x_flat = np.ascontiguousarray(x.reshape(T, HEAD_DIM))
    out = _run_device(x_flat, Rbd_dev)                     # [T, 128] fp32

    # Reproduce the reference exactly for any output feature whose R row has
    # non-finite entries: NaN rows give NaN output everywhere; inf rows are
    # recomputed with the same fp32 math the reference uses.
    if not finite_mask.all():
        row_nonfinite = ~np.isfinite(R).all(axis=2)        # [NBLK, 32]
        row_has_nan = np.isnan(R).any(axis=2)
        for k in range(NBLK):
            for i in range(BLOCK):
                if not row_nonfinite[k, i]:
                    continue
                col = k * BLOCK + i
                if row_has_nan[k, i]:
                    out[:, col] = np.float32(np.nan)
                else:  # inf but no nan: data-dependent, mirror in fp32
                    xb = x_flat[:, k * BLOCK:(k + 1) * BLOCK]
                    out[:, col] = (xb * R[k, i][None, :]).sum(axis=1,
                                                              dtype=np.float32)

    return out.reshape(B, SEQ, HEADS, HEAD_DIM)
